# revision 18
# baseline (speedup 1.0000x reference)
"""Trainium2 Bass kernel for nn_MultiHeadAttention_83777632076280.

Sharding: 8 cores = (half j in {0,1}) x (batch b in {0..3}).
Core c = j*4 + b computes, for batch b, heads j*8..j*8+7 (Q1/K1/V1/V2
projections restricted to those head columns), causal attention, the
corresponding fc (Wfc1 for j=0, Wfc2 for j=1), residual add and
layernorm -- i.e. the full out_j half [S, 512] plus softmax_attn for
its 8 heads. No cross-core communication is needed.

Everything on-device runs in bf16 matmuls (fp32 PSUM accumulation)
with fp32 softmax / layernorm arithmetic.
"""

import threading

import numpy as np
import ml_dtypes

import concourse.bass as bass
from concourse import bacc
import concourse.tile as tile
from concourse import mybir
from concourse.bass_utils import run_bass_kernel_spmd
from concourse.masks import make_identity

F32 = mybir.dt.float32
BF16 = mybir.dt.bfloat16
NPBF16 = ml_dtypes.bfloat16

B, S, D, H = 4, 1024, 1024, 16
HALF = D // 2            # 512
NH = 8                   # heads per core
DKH = 32                 # per-head q/k dim
DV = 64                  # per-head v dim (concat of V1h, V2h)
P = 128
KC = HALF // P           # 4 contraction chunks for projections / fc
TQ = S // P              # 8 token tiles
INV_SQRT_DKH = float(1.0 / np.sqrt(np.float32(DKH)))
NEG = -1e9


def _build_nc(causal: bool, stage: int = 4) -> bass.Bass:
    """stage: 1=loads+proj, 2=+scores/softmax/P, 3=+transpose/AV, 4=full (fc+LN)."""
    nc = bacc.Bacc(None)

    xqT = nc.declare_dram_parameter("xqT", [HALF, S], BF16, isOutput=False)
    xkT = nc.declare_dram_parameter("xkT", [HALF, S], BF16, isOutput=False)
    xv1T = nc.declare_dram_parameter("xv1T", [HALF, S], BF16, isOutput=False)
    xv2T = nc.declare_dram_parameter("xv2T", [HALF, S], BF16, isOutput=False)
    wq = nc.declare_dram_parameter("wq", [HALF, NH * DKH], BF16, isOutput=False)
    wk = nc.declare_dram_parameter("wk", [HALF, NH * DKH], BF16, isOutput=False)
    wv1 = nc.declare_dram_parameter("wv1", [HALF, NH * DKH], BF16, isOutput=False)
    wv2 = nc.declare_dram_parameter("wv2", [HALF, NH * DKH], BF16, isOutput=False)
    wfc = nc.declare_dram_parameter("wfc", [HALF, HALF], BF16, isOutput=False)
    res = nc.declare_dram_parameter("res", [S, HALF], F32, isOutput=False)
    lng = nc.declare_dram_parameter("lng", [P, HALF], F32, isOutput=False)
    lnb = nc.declare_dram_parameter("lnb", [P, HALF], F32, isOutput=False)
    if not causal:
        amask = nc.declare_dram_parameter("amask", [S, S], BF16, isOutput=False)

    out = nc.declare_dram_parameter("out", [S, HALF], F32, isOutput=True)
    p_out = nc.declare_dram_parameter("p_out", [NH, S, S], F32, isOutput=True)

    with tile.TileContext(nc) as tc:
        with (
            tc.tile_pool(name="const", bufs=1) as const,
            tc.tile_pool(name="pt", bufs=2) as pt_pool,
            tc.tile_pool(name="work", bufs=3) as work,
            tc.tile_pool(name="ln", bufs=2) as ln_pool,
            tc.tile_pool(name="stat", bufs=4) as stat,
            tc.tile_pool(name="psS", bufs=2, space="PSUM") as psS,
            tc.tile_pool(name="psmm", bufs=4, space="PSUM") as psmm,
        ):
            # ---- constants ----
            ident = const.tile([P, P], F32, tag="ident")
            make_identity(nc, ident)
            if causal:
                cmask = const.tile([P, P], F32, tag="cmask")
                nc.gpsimd.memset(cmask, 0.0)
                # keep 0.0 where (q - k) >= 0, else fill NEG
                nc.gpsimd.affine_select(
                    out=cmask,
                    in_=cmask,
                    compare_op=mybir.AluOpType.is_ge,
                    fill=NEG,
                    base=0,
                    pattern=[[-1, P]],
                    channel_multiplier=1,
                )
            eps_t = const.tile([P, 1], F32, tag="eps_t")
            nc.gpsimd.memset(eps_t, 1e-5)
            g_bc = const.tile([P, HALF], F32, tag="g_bc")
            b_bc = const.tile([P, HALF], F32, tag="b_bc")
            nc.sync.dma_start(g_bc, lng[:, :])
            nc.sync.dma_start(b_bc, lnb[:, :])

            # ---- input loads ----
            def load_T(ap, tag, fdim):
                t = const.tile([P, KC, fdim], BF16, tag=tag)
                nc.sync.dma_start(t, ap.rearrange("(c p) f -> p c f", p=P))
                return t

            xq_s = load_T(xqT, "xq_s", S)
            xk_s = load_T(xkT, "xk_s", S)
            xv1_s = load_T(xv1T, "xv1_s", S)
            xv2_s = load_T(xv2T, "xv2_s", S)
            wq_s = load_T(wq, "wq_s", NH * DKH)
            wk_s = load_T(wk, "wk_s", NH * DKH)
            wv1_s = load_T(wv1, "wv1_s", NH * DKH)
            wv2_s = load_T(wv2, "wv2_s", NH * DKH)
            wfc_s = load_T(wfc, "wfc_s", HALF)
            res_s = const.tile([P, TQ, HALF], F32, tag="res_s")
            nc.sync.dma_start(res_s, res.rearrange("(t p) f -> p t f", p=P))
            if not causal:
                am_s = const.tile([P, TQ, S], BF16, tag="am_s")
                nc.sync.dma_start(am_s, amask.rearrange("(t p) k -> p t k", p=P))

            # ---- persistent activations ----
            qt_s = const.tile([P, 2, S], BF16, tag="qt_s")    # QT: [outdim-part, chunk, tok]
            kt_s = const.tile([P, 2, S], BF16, tag="kt_s")
            vc_s = const.tile([P, TQ, HALF], BF16, tag="vc_s")  # V: [tok-part, toktile, 8*64]
            qkvT_s = const.tile([P, KC, S], BF16, tag="qkvT_s")

            # ---- phase 1: projections ----
            # QT/KT = W.T @ X.T  -> [256 outdims, S], outdims on partitions
            for w_s, x_s, dstT in ((wq_s, xq_s, qt_s), (wk_s, xk_s, kt_s)):
                for mc in range(2):
                    for t2 in range(2):
                        ps = psmm.tile([P, 512], F32, tag="mm")
                        for kc in range(KC):
                            nc.tensor.matmul(
                                ps,
                                lhsT=w_s[:, kc, mc * P:(mc + 1) * P],
                                rhs=x_s[:, kc, t2 * 512:(t2 + 1) * 512],
                                start=(kc == 0),
                                stop=(kc == KC - 1),
                            )
                        nc.scalar.copy(out=dstT[:, mc, t2 * 512:(t2 + 1) * 512], in_=ps)
            # V natural: [tok, 256] per half, interleaved into per-head [V1h|V2h]
            for t in range(TQ):
                for w_s, x_s, off in ((wv1_s, xv1_s, 0), (wv2_s, xv2_s, DKH)):
                    ps = psmm.tile([P, NH * DKH], F32, tag="mm")
                    for kc in range(KC):
                        nc.tensor.matmul(
                            ps,
                            lhsT=x_s[:, kc, t * P:(t + 1) * P],
                            rhs=w_s[:, kc, :],
                            start=(kc == 0),
                            stop=(kc == KC - 1),
                        )
                    nc.scalar.copy(
                        out=vc_s[:, t, :].rearrange("p (h d) -> p h d", d=DV)[:, :, off:off + DKH],
                        in_=ps.rearrange("p (h d) -> p h d", d=DKH),
                    )

            # ---- phase 2: attention per head ----
            for h in range(NH if stage >= 2 else 0):
                mc, prow = h // 4, 32 * (h % 4)
                ptb = pt_pool.tile([P, TQ, S], BF16, tag="ptb")  # P^T: [k-part, ktile, q]
                for qt in range(TQ):
                    kext = (qt + 1) * P if causal else S
                    sps = psS.tile([P, S], F32, tag="S")
                    for ks in range(0, kext, 512):
                        kw = min(512, kext - ks)
                        nc.tensor.matmul(
                            sps[:, ks:ks + kw],
                            lhsT=qt_s[prow:prow + 32, mc, qt * P:(qt + 1) * P],
                            rhs=kt_s[prow:prow + 32, mc, ks:ks + kw],
                            start=True,
                            stop=True,
                            tile_position=(prow, 0),
                        )
                    if causal:
                        nc.vector.tensor_tensor(
                            out=sps[:, qt * P:(qt + 1) * P],
                            in0=sps[:, qt * P:(qt + 1) * P],
                            in1=cmask,
                            op=mybir.AluOpType.add,
                        )
                    else:
                        nc.vector.tensor_tensor(
                            out=sps[:, :],
                            in0=sps[:, :],
                            in1=am_s[:, qt, :],
                            op=mybir.AluOpType.add,
                        )
                    e_t = work.tile([P, S], F32, tag="E")
                    s_sum = stat.tile([P, 1], F32, tag="ssum")
                    nc.scalar.activation(
                        out=e_t[:, :kext],
                        in_=sps[:, :kext],
                        func=mybir.ActivationFunctionType.Exp,
                        scale=INV_SQRT_DKH,
                        accum_out=s_sum,
                    )
                    r_t = stat.tile([P, 1], F32, tag="recip")
                    nc.vector.reciprocal(r_t, s_sum)
                    pr_t = work.tile([P, S], F32, tag="P")
                    nc.vector.tensor_scalar_mul(pr_t[:, :kext], e_t[:, :kext], r_t)
                    nc.sync.dma_start(
                        p_out[h, qt * P:(qt + 1) * P, :kext], pr_t[:, :kext]
                    )
                    # transpose P tile-by-tile into ptb (cast to bf16 on evac)
                    for kb in range((qt + 1 if causal else TQ) if stage >= 3 else 0):
                        trp = psmm.tile([P, P], F32, tag="mm")
                        nc.tensor.transpose(trp, pr_t[:, kb * P:(kb + 1) * P], ident)
                        if kb % 2 == 0:
                            nc.scalar.copy(out=ptb[:, kb, qt * P:(qt + 1) * P], in_=trp)
                        else:
                            nc.vector.tensor_copy(out=ptb[:, kb, qt * P:(qt + 1) * P], in_=trp)
                # AV: O^T[64, q] accumulated over k tiles; feeds qkvT directly
                for qc in range(2 if stage >= 3 else 0):
                    q0 = qc * 512
                    ot = psmm.tile([64, 512], F32, tag="mm")
                    if causal:
                        kts = [kt for kt in range(TQ) if kt * P < q0 + 512]
                    else:
                        kts = list(range(TQ))
                    for i, kt in enumerate(kts):
                        qoff = max(q0, kt * P) if causal else q0
                        nc.tensor.matmul(
                            ot[:, qoff - q0:],
                            lhsT=vc_s[:, kt, h * DV:(h + 1) * DV],
                            rhs=ptb[:, kt, qoff:q0 + 512],
                            start=(i == 0),
                            stop=(i == len(kts) - 1),
                        )
                    nc.scalar.copy(
                        out=qkvT_s[64 * (h % 2):64 * (h % 2) + 64, h // 2, q0:q0 + 512],
                        in_=ot,
                    )

            # ---- phase 3: fc + residual + layernorm ----
            if stage < 4:
                # still produce "out" so outputs are all written
                for t in range(TQ):
                    o_t = ln_pool.tile([P, HALF], F32, tag="o")
                    nc.vector.tensor_copy(out=o_t, in_=res_s[:, t, :])
                    nc.sync.dma_start(out[t * P:(t + 1) * P, :], o_t)
            for t in range(TQ if stage >= 4 else 0):
                fps = psmm.tile([P, HALF], F32, tag="mm")
                for kc in range(KC):
                    nc.tensor.matmul(
                        fps,
                        lhsT=qkvT_s[:, kc, t * P:(t + 1) * P],
                        rhs=wfc_s[:, kc, :],
                        start=(kc == 0),
                        stop=(kc == KC - 1),
                    )
                x_t = ln_pool.tile([P, HALF], F32, tag="x")
                sum_t = stat.tile([P, 1], F32, tag="lnsum")
                nc.vector.tensor_tensor(out=x_t, in0=fps, in1=res_s[:, t, :], op=mybir.AluOpType.add)
                nc.vector.tensor_reduce(
                    out=sum_t, in_=x_t, axis=mybir.AxisListType.X, op=mybir.AluOpType.add
                )
                sq_t = ln_pool.tile([P, HALF], F32, tag="sq")
                ssq_t = stat.tile([P, 1], F32, tag="lnssq")
                nc.scalar.activation(
                    out=sq_t,
                    in_=x_t,
                    func=mybir.ActivationFunctionType.Square,
                    accum_out=ssq_t,
                )
                mu_t = stat.tile([P, 1], F32, tag="mu")
                nc.scalar.mul(mu_t, sum_t, 1.0 / HALF)
                musq_t = stat.tile([P, 1], F32, tag="musq")
                nc.vector.tensor_tensor(out=musq_t, in0=mu_t, in1=mu_t, op=mybir.AluOpType.mult)
                var_t = stat.tile([P, 1], F32, tag="var")
                nc.vector.tensor_scalar(var_t, ssq_t, 1.0 / HALF, None, mybir.AluOpType.mult)
                nc.vector.tensor_tensor(out=var_t, in0=var_t, in1=musq_t, op=mybir.AluOpType.subtract)
                sd_t = stat.tile([P, 1], F32, tag="sd")
                nc.scalar.activation(
                    out=sd_t, in_=var_t,
                    func=mybir.ActivationFunctionType.Sqrt, bias=eps_t[:, :],
                )
                rstd_t = stat.tile([P, 1], F32, tag="rstd")
                nc.vector.reciprocal(rstd_t, sd_t)
                y_t = ln_pool.tile([P, HALF], F32, tag="y")
                nc.vector.tensor_scalar(y_t, x_t, mu_t, None, mybir.AluOpType.subtract)
                nc.vector.tensor_scalar_mul(y_t, y_t, rstd_t)
                o_t = ln_pool.tile([P, HALF], F32, tag="o")
                nc.vector.tensor_tensor(out=o_t, in0=y_t, in1=g_bc, op=mybir.AluOpType.mult)
                nc.vector.tensor_tensor(out=o_t, in0=o_t, in1=b_bc, op=mybir.AluOpType.add)
                nc.sync.dma_start(out[t * P:(t + 1) * P, :], o_t)

    nc.finalize()
    return nc


_nc_lock = threading.Lock()
_nc_cache: dict[bool, bass.Bass] = {}


def _get_nc(causal: bool) -> bass.Bass:
    with _nc_lock:
        if causal not in _nc_cache:
            _nc_cache[causal] = _build_nc(causal)
        return _nc_cache[causal]


def _bf(x):
    return np.ascontiguousarray(np.asarray(x, dtype=np.float32)).astype(NPBF16)


def _prep_in_maps(inputs, causal):
    input_Q = np.asarray(inputs["input_Q"], dtype=np.float32)
    input_K = np.asarray(inputs["input_K"], dtype=np.float32)
    input_V = np.asarray(inputs["input_V"], dtype=np.float32)
    attn_mask = np.asarray(inputs["attn_mask"])
    WQ1 = np.asarray(inputs["WQ1"], dtype=np.float32)
    WK1 = np.asarray(inputs["WK1"], dtype=np.float32)
    WV1 = np.asarray(inputs["WV1"], dtype=np.float32)
    WV2 = np.asarray(inputs["WV2"], dtype=np.float32)
    Wfc1 = np.asarray(inputs["Wfc1"], dtype=np.float32)
    Wfc2 = np.asarray(inputs["Wfc2"], dtype=np.float32)
    ln_g = np.asarray(inputs["ln_g"], dtype=np.float32)
    ln_b = np.asarray(inputs["ln_b"], dtype=np.float32)

    lng = np.ascontiguousarray(np.broadcast_to(ln_g[None, :], (P, HALF)))
    lnb = np.ascontiguousarray(np.broadcast_to(ln_b[None, :], (P, HALF)))
    wfcs = (_bf(Wfc1), _bf(Wfc2))

    in_maps = []
    for c in range(8):
        j, b = c // 4, c % 4
        cols = slice(j * NH * DKH, (j + 1) * NH * DKH)
        m = {
            "xqT": _bf(input_Q[b][:, :HALF].T),
            "xkT": _bf(input_K[b][:, :HALF].T),
            "xv1T": _bf(input_V[b][:, :HALF].T),
            "xv2T": _bf(input_V[b][:, HALF:].T),
            "wq": _bf(WQ1[:, cols]),
            "wk": _bf(WK1[:, cols]),
            "wv1": _bf(WV1[:, cols]),
            "wv2": _bf(WV2[:, cols]),
            "wfc": wfcs[j],
            "res": np.ascontiguousarray(input_Q[b][:, j * HALF:(j + 1) * HALF]),
            "lng": lng,
            "lnb": lnb,
        }
        if not causal:
            m["amask"] = (attn_mask[b].astype(np.float32) * np.float32(NEG)).astype(NPBF16)
        in_maps.append(m)
    return in_maps


def kernel(**inputs):
    attn_mask = np.asarray(inputs["attn_mask"])
    triu = np.triu(np.ones((S, S), dtype=bool), k=1)
    causal = all(np.array_equal(attn_mask[b], triu) for b in range(B))

    nc = _get_nc(causal)
    in_maps = _prep_in_maps(inputs, causal)

    results = run_bass_kernel_spmd(nc, in_maps, list(range(8))).results

    output = np.empty((B, S, D), dtype=np.float32)
    softmax_attn = np.empty((B, H, S, S), dtype=np.float32)
    for c in range(8):
        j, b = c // 4, c % 4
        output[b][:, j * HALF:(j + 1) * HALF] = results[c]["out"]
        softmax_attn[b, j * NH:(j + 1) * NH] = results[c]["p_out"]
    return output, softmax_attn


# revision 51
# speedup vs baseline: 1.5420x; 1.5420x over previous
"""Trainium2 Bass kernel for nn_MultiHeadAttention_83777632076280.

Sharding: 8 cores = (half j in {0,1}) x (batch b in {0..3}).
Core c = j*4 + b computes, for batch b, heads j*8..j*8+7 (Q1/K1/V1/V2
projections restricted to those head columns), causal attention, the
corresponding fc (Wfc1 for j=0, Wfc2 for j=1), residual add and
layernorm -- i.e. the full out_j half [S, 512] plus softmax_attn for
its 8 heads. No cross-core communication is needed.

Everything on-device runs in bf16 matmuls (fp32 PSUM accumulation)
with fp32 softmax / layernorm arithmetic.
"""

import threading

import numpy as np
import ml_dtypes

import concourse.bass as bass
from concourse import bacc
import concourse.tile as tile
from concourse import mybir
from concourse.bass_utils import run_bass_kernel_spmd
from concourse.masks import make_identity

F32 = mybir.dt.float32
BF16 = mybir.dt.bfloat16
NPBF16 = ml_dtypes.bfloat16

B, S, D, H = 4, 1024, 1024, 16
HALF = D // 2            # 512
NH = 8                   # heads per core
DKH = 32                 # per-head q/k dim
DV = 64                  # per-head v dim (concat of V1h, V2h)
P = 128
KC = HALF // P           # 4 contraction chunks for projections / fc
TQ = S // P              # 8 token tiles
INV_SQRT_DKH = float(1.0 / np.sqrt(np.float32(DKH)))
NEG = -1e9


def _build_nc(causal: bool, stage: int = 4, unit_ln: bool = False,
              risky_ops: bool = True) -> bass.Bass:
    """stage: 1=loads+proj, 2=+scores/softmax/P, 3=+transpose/AV, 4=full (fc+LN)."""
    nc = bacc.Bacc(None)

    xqT = nc.declare_dram_parameter("xqT", [HALF, S], BF16, isOutput=False)
    xkT = nc.declare_dram_parameter("xkT", [HALF, S], BF16, isOutput=False)
    xv1T = nc.declare_dram_parameter("xv1T", [HALF, S], BF16, isOutput=False)
    xv2T = nc.declare_dram_parameter("xv2T", [HALF, S], BF16, isOutput=False)
    wq = nc.declare_dram_parameter("wq", [HALF, NH * DKH], BF16, isOutput=False)
    wk = nc.declare_dram_parameter("wk", [HALF, NH * DKH], BF16, isOutput=False)
    wv1 = nc.declare_dram_parameter("wv1", [HALF, NH * DKH], BF16, isOutput=False)
    wv2 = nc.declare_dram_parameter("wv2", [HALF, NH * DKH], BF16, isOutput=False)
    wfc = nc.declare_dram_parameter("wfc", [HALF, HALF], BF16, isOutput=False)
    res = nc.declare_dram_parameter("res", [S, HALF], F32, isOutput=False)
    lng = nc.declare_dram_parameter("lng", [P, HALF], F32, isOutput=False)
    lnb = nc.declare_dram_parameter("lnb", [P, HALF], F32, isOutput=False)
    if not causal:
        amask = nc.declare_dram_parameter("amask", [S, S], BF16, isOutput=False)

    out = nc.declare_dram_parameter("out", [S, HALF], F32, isOutput=True)
    p_out = nc.declare_dram_parameter("p_out", [NH, S, S], F32, isOutput=True)

    with tile.TileContext(nc) as tc:
        with (
            tc.tile_pool(name="const", bufs=1) as const,
            tc.tile_pool(name="pt", bufs=2) as pt_pool,
            tc.tile_pool(name="work", bufs=4) as work,
            tc.tile_pool(name="ln", bufs=3) as ln_pool,
            tc.tile_pool(name="stat", bufs=6) as stat,
            tc.tile_pool(name="psS", bufs=4, space="PSUM") as psS,
            tc.tile_pool(name="psmm", bufs=2, space="PSUM") as psmm,
            tc.tile_pool(name="psTr", bufs=2, space="PSUM") as psTr,
        ):
            # ---- constants ----
            ident = const.tile([P, P], F32, tag="ident")
            make_identity(nc, ident)
            ident_r = ident.bitcast(mybir.dt.float32r)
            identb = const.tile([P, P], BF16, tag="identb")
            make_identity(nc, identb)
            if causal:
                # additive causal mask for the diagonal block, bf16 so it can
                # be accumulated into the scores PSUM via a PE matmul
                # (identb.T @ cmask) instead of a DVE pass.
                cmask = const.tile([P, P], BF16, tag="cmask")
                nc.gpsimd.memset(cmask, 0.0)
                # keep 0.0 where (q - k) >= 0, else fill NEG
                nc.gpsimd.affine_select(
                    out=cmask,
                    in_=cmask,
                    compare_op=mybir.AluOpType.is_ge,
                    fill=NEG,
                    base=0,
                    pattern=[[-1, P]],
                    channel_multiplier=1,
                )
            eps_t = const.tile([P, 1], F32, tag="eps_t")
            nc.gpsimd.memset(eps_t, 1e-5)
            if not unit_ln:
                g_bc = const.tile([P, HALF], F32, tag="g_bc")
                b_bc = const.tile([P, HALF], F32, tag="b_bc")
                nc.sync.dma_start(g_bc, lng[:, :])
                nc.sync.dma_start(b_bc, lnb[:, :])

            # ---- input loads (chunked per contraction slice so the first
            # projection matmuls can start before the whole tensor lands) ----
            def load_T(ap, tag, fdim, chunked=False):
                t = const.tile([P, KC, fdim], BF16, tag=tag)
                src = ap.rearrange("(c p) f -> p c f", p=P)
                if chunked:
                    for c in range(KC):
                        nc.sync.dma_start(t[:, c], src[:, c])
                else:
                    nc.sync.dma_start(t, src)
                return t

            wq_s = load_T(wq, "wq_s", NH * DKH)
            wk_s = load_T(wk, "wk_s", NH * DKH)
            wv1_s = load_T(wv1, "wv1_s", NH * DKH)
            wv2_s = load_T(wv2, "wv2_s", NH * DKH)
            wfc_s = load_T(wfc, "wfc_s", HALF)
            xq_s = load_T(xqT, "xq_s", S, chunked=True)
            xk_s = load_T(xkT, "xk_s", S, chunked=True)
            xv1_s = load_T(xv1T, "xv1_s", S, chunked=True)
            xv2_s = load_T(xv2T, "xv2_s", S, chunked=True)
            res_s = const.tile([P, TQ, HALF], F32, tag="res_s")
            nc.sync.dma_start(res_s, res.rearrange("(t p) f -> p t f", p=P))
            if not causal:
                am_s = const.tile([P, TQ, S], BF16, tag="am_s")
                nc.sync.dma_start(am_s, amask.rearrange("(t p) k -> p t k", p=P))

            # ---- persistent activations ----
            qt_s = const.tile([P, 2, S], BF16, tag="qt_s")    # QT: [outdim-part, chunk, tok]
            kt_s = const.tile([P, 2, S], BF16, tag="kt_s")
            vc_s = const.tile([P, TQ, HALF], BF16, tag="vc_s")  # V: [tok-part, toktile, 8*64]
            qkvT_s = const.tile([P, KC, S], BF16, tag="qkvT_s")

            # ---- phase 1: projections ----
            # QT/KT = W.T @ X.T  -> [256 outdims, S], outdims on partitions
            for w_s, x_s, dstT in ((wq_s, xq_s, qt_s), (wk_s, xk_s, kt_s)):
                for mc in range(2):
                    for t2 in range(2):
                        ps = psmm.tile([P, 512], F32, tag="mm")
                        for kc in range(KC):
                            nc.tensor.matmul(
                                ps,
                                lhsT=w_s[:, kc, mc * P:(mc + 1) * P],
                                rhs=x_s[:, kc, t2 * 512:(t2 + 1) * 512],
                                start=(kc == 0),
                                stop=(kc == KC - 1),
                            )
                        nc.any.tensor_copy(out=dstT[:, mc, t2 * 512:(t2 + 1) * 512], in_=ps)
            # V natural: [tok, 256] per half, interleaved into per-head [V1h|V2h]
            for t in range(TQ):
                for w_s, x_s, off in ((wv1_s, xv1_s, 0), (wv2_s, xv2_s, DKH)):
                    ps = psmm.tile([P, NH * DKH], F32, tag="mm")
                    for kc in range(KC):
                        nc.tensor.matmul(
                            ps,
                            lhsT=x_s[:, kc, t * P:(t + 1) * P],
                            rhs=w_s[:, kc, :],
                            start=(kc == 0),
                            stop=(kc == KC - 1),
                        )
                    dst_v = vc_s[:, t, :].rearrange("p (h d) -> p h d", d=DV)[:, :, off:off + DKH]
                    src_v = ps.rearrange("p (h d) -> p h d", d=DKH)
                    nc.any.tensor_copy(out=dst_v, in_=src_v)

            # ---- phase 2: attention per head ----
            for h in range(NH if stage >= 2 else 0):
                mc, prow = h // 4, 32 * (h % 4)
                ptb = pt_pool.tile([P, TQ, S], BF16, tag="ptb")  # P^T: [k-part, ktile, q]
                for qt in range(TQ):
                    kext = (qt + 1) * P if causal else S
                    lhsT_q = qt_s[prow:prow + 32, mc, qt * P:(qt + 1) * P]
                    e_t = work.tile([P, S], F32, tag="E")
                    nchunk = (kext + 511) // 512
                    s_parts = stat.tile([P, 2], F32, tag="sparts")
                    for ci in range(nchunk):
                        ks = ci * 512
                        kw = min(512, kext - ks)
                        sps = psS.tile([P, 512], F32, tag="S")
                        # diagonal (causal-masked) block is the last 128 cols
                        kreg = kw - P if causal and ci == nchunk - 1 else kw
                        if kreg > 0:
                            nc.tensor.matmul(
                                sps[:, :kreg],
                                lhsT=lhsT_q,
                                rhs=kt_s[prow:prow + 32, mc, ks:ks + kreg],
                                start=True,
                                stop=True,
                                tile_position=(prow, 0),
                            )
                        if causal and ci == nchunk - 1:
                            nc.tensor.matmul(
                                sps[:, kreg:kw],
                                lhsT=lhsT_q,
                                rhs=kt_s[prow:prow + 32, mc, ks + kreg:ks + kw],
                                start=True,
                                stop=False,
                                tile_position=(prow, 0),
                            )
                            nc.tensor.matmul(
                                sps[:, kreg:kw],
                                lhsT=identb,
                                rhs=cmask,
                                start=False,
                                stop=True,
                            )
                        if not causal:
                            nc.vector.tensor_tensor(
                                out=sps[:, :kw],
                                in0=sps[:, :kw],
                                in1=am_s[:, qt, ks:ks + kw],
                                op=mybir.AluOpType.add,
                            )
                        nc.scalar.activation(
                            out=e_t[:, ks:ks + kw],
                            in_=sps[:, :kw],
                            func=mybir.ActivationFunctionType.Exp,
                            scale=INV_SQRT_DKH,
                            accum_out=s_parts[:, ci:ci + 1],
                        )
                    r_t = stat.tile([P, 1], F32, tag="recip")
                    if nchunk > 1:
                        s_sum = stat.tile([P, 1], F32, tag="ssum")
                        nc.vector.tensor_tensor(
                            out=s_sum, in0=s_parts[:, 0:1], in1=s_parts[:, 1:2],
                            op=mybir.AluOpType.add,
                        )
                    else:
                        s_sum = s_parts[:, 0:1]
                    nc.vector.reciprocal(r_t, s_sum)
                    pr_t = work.tile([P, S], F32, tag="P")
                    nc.gpsimd.tensor_scalar_mul(pr_t[:, :kext], e_t[:, :kext], r_t)
                    nc.sync.dma_start(
                        p_out[h, qt * P:(qt + 1) * P, :kext], pr_t[:, :kext]
                    )
                    # normalized P in bf16 (DVE 2x mode) for the transpose+AV path
                    pb_t = work.tile([P, S], BF16, tag="Pb")
                    nc.vector.tensor_scalar_mul(pb_t[:, :kext], e_t[:, :kext], r_t)
                    # transpose P into ptb, up to 8 blocks per bf16 PSUM tile,
                    # one wide evac copy per qt
                    nkb = (qt + 1 if causal else TQ) if stage >= 3 else 0
                    if nkb > 0:
                        trp = psTr.tile([P, S], BF16, tag="mmb")
                        for kb in range(nkb):
                            nc.tensor.transpose(
                                trp[:, kb * P:(kb + 1) * P],
                                pb_t[:, kb * P:(kb + 1) * P],
                                identb,
                            )
                        src = trp[:, :nkb * P].rearrange("p (b q) -> p b q", q=P)
                        dst = ptb[:, 0:nkb, qt * P:(qt + 1) * P]
                        nc.any.tensor_copy(out=dst, in_=src)
                # AV: O^T[64, q] accumulated over k tiles; feeds qkvT directly
                for qc in range(2 if stage >= 3 else 0):
                    q0 = qc * 512
                    ot = psmm.tile([64, 512], F32, tag="mm")
                    if causal:
                        kts = [kt for kt in range(TQ) if kt * P < q0 + 512]
                    else:
                        kts = list(range(TQ))
                    for i, kt in enumerate(kts):
                        qoff = max(q0, kt * P) if causal else q0
                        nc.tensor.matmul(
                            ot[:, qoff - q0:],
                            lhsT=vc_s[:, kt, h * DV:(h + 1) * DV],
                            rhs=ptb[:, kt, qoff:q0 + 512],
                            start=(i == 0),
                            stop=(i == len(kts) - 1),
                        )
                    nc.any.tensor_copy(
                        out=qkvT_s[64 * (h % 2):64 * (h % 2) + 64, h // 2, q0:q0 + 512],
                        in_=ot,
                    )

            # ---- phase 3: fc + residual + layernorm ----
            if stage < 4:
                # still produce "out" so outputs are all written
                for t in range(TQ):
                    o_t = ln_pool.tile([P, HALF], F32, tag="o")
                    nc.vector.tensor_copy(out=o_t, in_=res_s[:, t, :])
                    nc.sync.dma_start(out[t * P:(t + 1) * P, :], o_t)
            for t in range(TQ if stage >= 4 else 0):
                fps = psmm.tile([P, HALF], F32, tag="mm")
                for kc in range(KC):
                    nc.tensor.matmul(
                        fps,
                        lhsT=qkvT_s[:, kc, t * P:(t + 1) * P],
                        rhs=wfc_s[:, kc, :],
                        start=(kc == 0),
                        stop=(kc == KC - 1),
                    )
                x_t = ln_pool.tile([P, HALF], F32, tag="x")
                sum_t = stat.tile([P, 1], F32, tag="lnsum")
                if risky_ops:
                    # one DVE pass: x = fps + res, sum_t = rowsum(x)
                    nc.vector.scalar_tensor_tensor(
                        out=x_t, in0=fps, scalar=1.0, in1=res_s[:, t, :],
                        op0=mybir.AluOpType.mult, op1=mybir.AluOpType.add,
                        accum_out=sum_t,
                    )
                else:
                    nc.vector.tensor_tensor(out=x_t, in0=fps, in1=res_s[:, t, :], op=mybir.AluOpType.add)
                    nc.vector.tensor_reduce(
                        out=sum_t, in_=x_t, axis=mybir.AxisListType.X, op=mybir.AluOpType.add
                    )
                sq_t = ln_pool.tile([P, HALF], F32, tag="sq")
                ssq_t = stat.tile([P, 1], F32, tag="lnssq")
                nc.scalar.activation(
                    out=sq_t,
                    in_=x_t,
                    func=mybir.ActivationFunctionType.Square,
                    accum_out=ssq_t,
                )
                mu_t = stat.tile([P, 1], F32, tag="mu")
                nc.scalar.mul(mu_t, sum_t, 1.0 / HALF)
                musq_t = stat.tile([P, 1], F32, tag="musq")
                nc.vector.tensor_tensor(out=musq_t, in0=mu_t, in1=mu_t, op=mybir.AluOpType.mult)
                var_t = stat.tile([P, 1], F32, tag="var")
                nc.vector.tensor_scalar(var_t, ssq_t, 1.0 / HALF, None, mybir.AluOpType.mult)
                nc.vector.tensor_tensor(out=var_t, in0=var_t, in1=musq_t, op=mybir.AluOpType.subtract)
                sd_t = stat.tile([P, 1], F32, tag="sd")
                nc.scalar.activation(
                    out=sd_t, in_=var_t,
                    func=mybir.ActivationFunctionType.Sqrt, bias=eps_t[:, :],
                )
                rstd_t = stat.tile([P, 1], F32, tag="rstd")
                nc.vector.reciprocal(rstd_t, sd_t)
                o_t = ln_pool.tile([P, HALF], F32, tag="o")
                y_dst = o_t if unit_ln else ln_pool.tile([P, HALF], F32, tag="y")
                if risky_ops:
                    nc.vector.tensor_scalar(
                        y_dst, x_t, mu_t, rstd_t,
                        mybir.AluOpType.subtract, mybir.AluOpType.mult,
                    )
                else:
                    nc.vector.tensor_scalar(y_dst, x_t, mu_t, None, mybir.AluOpType.subtract)
                    nc.vector.tensor_scalar_mul(y_dst, y_dst, rstd_t)
                if not unit_ln:
                    nc.vector.tensor_tensor(out=o_t, in0=y_dst, in1=g_bc, op=mybir.AluOpType.mult)
                    nc.vector.tensor_tensor(out=o_t, in0=o_t, in1=b_bc, op=mybir.AluOpType.add)
                nc.sync.dma_start(out[t * P:(t + 1) * P, :], o_t)

    nc.finalize()
    return nc


_nc_lock = threading.Lock()
_nc_cache: dict[tuple, bass.Bass] = {}


def _get_nc(causal: bool, unit_ln: bool = False) -> bass.Bass:
    key = (causal, unit_ln)
    with _nc_lock:
        if key not in _nc_cache:
            _nc_cache[key] = _build_nc(causal, unit_ln=unit_ln)
        return _nc_cache[key]


def _bf(x):
    return np.ascontiguousarray(np.asarray(x, dtype=np.float32)).astype(NPBF16)


def _prep_in_maps(inputs, causal):
    input_Q = np.asarray(inputs["input_Q"], dtype=np.float32)
    input_K = np.asarray(inputs["input_K"], dtype=np.float32)
    input_V = np.asarray(inputs["input_V"], dtype=np.float32)
    attn_mask = np.asarray(inputs["attn_mask"])
    WQ1 = np.asarray(inputs["WQ1"], dtype=np.float32)
    WK1 = np.asarray(inputs["WK1"], dtype=np.float32)
    WV1 = np.asarray(inputs["WV1"], dtype=np.float32)
    WV2 = np.asarray(inputs["WV2"], dtype=np.float32)
    Wfc1 = np.asarray(inputs["Wfc1"], dtype=np.float32)
    Wfc2 = np.asarray(inputs["Wfc2"], dtype=np.float32)
    ln_g = np.asarray(inputs["ln_g"], dtype=np.float32)
    ln_b = np.asarray(inputs["ln_b"], dtype=np.float32)

    lng = np.ascontiguousarray(np.broadcast_to(ln_g[None, :], (P, HALF)))
    lnb = np.ascontiguousarray(np.broadcast_to(ln_b[None, :], (P, HALF)))
    wfcs = (_bf(Wfc1), _bf(Wfc2))

    in_maps = []
    for c in range(8):
        j, b = c // 4, c % 4
        cols = slice(j * NH * DKH, (j + 1) * NH * DKH)
        m = {
            "xqT": _bf(input_Q[b][:, :HALF].T),
            "xkT": _bf(input_K[b][:, :HALF].T),
            "xv1T": _bf(input_V[b][:, :HALF].T),
            "xv2T": _bf(input_V[b][:, HALF:].T),
            "wq": _bf(WQ1[:, cols]),
            "wk": _bf(WK1[:, cols]),
            "wv1": _bf(WV1[:, cols]),
            "wv2": _bf(WV2[:, cols]),
            "wfc": wfcs[j],
            "res": np.ascontiguousarray(input_Q[b][:, j * HALF:(j + 1) * HALF]),
            "lng": lng,
            "lnb": lnb,
        }
        if not causal:
            m["amask"] = (attn_mask[b].astype(np.float32) * np.float32(NEG)).astype(NPBF16)
        in_maps.append(m)
    return in_maps


def kernel(**inputs):
    attn_mask = np.asarray(inputs["attn_mask"])
    triu = np.triu(np.ones((S, S), dtype=bool), k=1)
    causal = all(np.array_equal(attn_mask[b], triu) for b in range(B))
    unit_ln = bool(
        np.all(np.asarray(inputs["ln_g"], dtype=np.float32) == 1.0)
        and np.all(np.asarray(inputs["ln_b"], dtype=np.float32) == 0.0)
    )

    nc = _get_nc(causal, unit_ln)
    in_maps = _prep_in_maps(inputs, causal)

    results = run_bass_kernel_spmd(nc, in_maps, list(range(8))).results

    output = np.empty((B, S, D), dtype=np.float32)
    softmax_attn = np.empty((B, H, S, S), dtype=np.float32)
    for c in range(8):
        j, b = c // 4, c % 4
        output[b][:, j * HALF:(j + 1) * HALF] = results[c]["out"]
        softmax_attn[b, j * NH:(j + 1) * NH] = results[c]["p_out"]
    return output, softmax_attn


# revision 52
# speedup vs baseline: 1.6625x; 1.0781x over previous
"""Trainium2 Bass kernel for nn_MultiHeadAttention_83777632076280.

Sharding: 8 cores = (half j in {0,1}) x (batch b in {0..3}).
Core c = j*4 + b computes, for batch b, heads j*8..j*8+7 (Q1/K1/V1/V2
projections restricted to those head columns), causal attention, the
corresponding fc (Wfc1 for j=0, Wfc2 for j=1), residual add and
layernorm -- i.e. the full out_j half [S, 512] plus softmax_attn for
its 8 heads. No cross-core communication is needed.

Everything on-device runs in bf16 matmuls (fp32 PSUM accumulation)
with fp32 softmax / layernorm arithmetic.
"""

import threading

import numpy as np
import ml_dtypes

import concourse.bass as bass
from concourse import bacc
import concourse.tile as tile
from concourse import mybir
from concourse.bass_utils import run_bass_kernel_spmd
from concourse.masks import make_identity

F32 = mybir.dt.float32
BF16 = mybir.dt.bfloat16
NPBF16 = ml_dtypes.bfloat16

B, S, D, H = 4, 1024, 1024, 16
HALF = D // 2            # 512
NH = 8                   # heads per core
DKH = 32                 # per-head q/k dim
DV = 64                  # per-head v dim (concat of V1h, V2h)
P = 128
KC = HALF // P           # 4 contraction chunks for projections / fc
TQ = S // P              # 8 token tiles
INV_SQRT_DKH = float(1.0 / np.sqrt(np.float32(DKH)))
NEG = -1e9


def _build_nc(causal: bool, stage: int = 4, unit_ln: bool = False,
              risky_ops: bool = True) -> bass.Bass:
    """stage: 1=loads+proj, 2=+scores/softmax/P, 3=+transpose/AV, 4=full (fc+LN)."""
    nc = bacc.Bacc(None)

    xqT = nc.declare_dram_parameter("xqT", [HALF, S], BF16, isOutput=False)
    xkT = nc.declare_dram_parameter("xkT", [HALF, S], BF16, isOutput=False)
    xv1T = nc.declare_dram_parameter("xv1T", [HALF, S], BF16, isOutput=False)
    xv2T = nc.declare_dram_parameter("xv2T", [HALF, S], BF16, isOutput=False)
    wq = nc.declare_dram_parameter("wq", [HALF, NH * DKH], BF16, isOutput=False)
    wk = nc.declare_dram_parameter("wk", [HALF, NH * DKH], BF16, isOutput=False)
    wv1 = nc.declare_dram_parameter("wv1", [HALF, NH * DKH], BF16, isOutput=False)
    wv2 = nc.declare_dram_parameter("wv2", [HALF, NH * DKH], BF16, isOutput=False)
    wfc = nc.declare_dram_parameter("wfc", [HALF, HALF], BF16, isOutput=False)
    res = nc.declare_dram_parameter("res", [S, HALF], F32, isOutput=False)
    lng = nc.declare_dram_parameter("lng", [P, HALF], F32, isOutput=False)
    lnb = nc.declare_dram_parameter("lnb", [P, HALF], F32, isOutput=False)
    if not causal:
        amask = nc.declare_dram_parameter("amask", [S, S], BF16, isOutput=False)

    out = nc.declare_dram_parameter("out", [S, HALF], F32, isOutput=True)
    p_out = nc.declare_dram_parameter("p_out", [NH, S, S], F32, isOutput=True)

    with tile.TileContext(nc) as tc:
        with (
            tc.tile_pool(name="const", bufs=1) as const,
            tc.tile_pool(name="pt", bufs=2) as pt_pool,
            tc.tile_pool(name="work", bufs=5) as work,
            tc.tile_pool(name="ln", bufs=3) as ln_pool,
            tc.tile_pool(name="stat", bufs=6) as stat,
            tc.tile_pool(name="psS", bufs=4, space="PSUM") as psS,
            tc.tile_pool(name="psmm", bufs=2, space="PSUM") as psmm,
            tc.tile_pool(name="psTr", bufs=2, space="PSUM") as psTr,
        ):
            # ---- constants ----
            ident = const.tile([P, P], F32, tag="ident")
            make_identity(nc, ident)
            ident_r = ident.bitcast(mybir.dt.float32r)
            identb = const.tile([P, P], BF16, tag="identb")
            make_identity(nc, identb)
            if causal:
                # additive causal mask for the diagonal block, bf16 so it can
                # be accumulated into the scores PSUM via a PE matmul
                # (identb.T @ cmask) instead of a DVE pass.
                cmask = const.tile([P, P], BF16, tag="cmask")
                nc.gpsimd.memset(cmask, 0.0)
                # keep 0.0 where (q - k) >= 0, else fill NEG
                nc.gpsimd.affine_select(
                    out=cmask,
                    in_=cmask,
                    compare_op=mybir.AluOpType.is_ge,
                    fill=NEG,
                    base=0,
                    pattern=[[-1, P]],
                    channel_multiplier=1,
                )
            eps_t = const.tile([P, 1], F32, tag="eps_t")
            nc.gpsimd.memset(eps_t, 1e-5)

            # ---- input loads (chunked per contraction slice so the first
            # projection matmuls can start before the whole tensor lands) ----
            def load_T(ap, tag, fdim, chunked=False):
                t = const.tile([P, KC, fdim], BF16, tag=tag)
                src = ap.rearrange("(c p) f -> p c f", p=P)
                if chunked:
                    for c in range(KC):
                        nc.sync.dma_start(t[:, c], src[:, c])
                else:
                    nc.sync.dma_start(t, src)
                return t

            wq_s = load_T(wq, "wq_s", NH * DKH)
            wk_s = load_T(wk, "wk_s", NH * DKH)
            wv1_s = load_T(wv1, "wv1_s", NH * DKH)
            wv2_s = load_T(wv2, "wv2_s", NH * DKH)
            xq_s = load_T(xqT, "xq_s", S, chunked=True)
            xk_s = load_T(xkT, "xk_s", S, chunked=True)
            xv1_s = load_T(xv1T, "xv1_s", S, chunked=True)
            xv2_s = load_T(xv2T, "xv2_s", S, chunked=True)
            if not causal:
                am_s = const.tile([P, TQ, S], BF16, tag="am_s")
                nc.sync.dma_start(am_s, amask.rearrange("(t p) k -> p t k", p=P))

            # ---- persistent activations ----
            qt_s = const.tile([P, 2, S], BF16, tag="qt_s")    # QT: [outdim-part, chunk, tok]
            kt_s = const.tile([P, 2, S], BF16, tag="kt_s")
            vc_s = const.tile([P, TQ, HALF], BF16, tag="vc_s")  # V: [tok-part, toktile, 8*64]
            qkvT_s = const.tile([P, KC, S], BF16, tag="qkvT_s")

            # ---- phase 1: projections ----
            # QT/KT = W.T @ X.T  -> [256 outdims, S], outdims on partitions
            for w_s, x_s, dstT in ((wq_s, xq_s, qt_s), (wk_s, xk_s, kt_s)):
                for mc in range(2):
                    for t2 in range(2):
                        ps = psmm.tile([P, 512], F32, tag="mm")
                        for kc in range(KC):
                            nc.tensor.matmul(
                                ps,
                                lhsT=w_s[:, kc, mc * P:(mc + 1) * P],
                                rhs=x_s[:, kc, t2 * 512:(t2 + 1) * 512],
                                start=(kc == 0),
                                stop=(kc == KC - 1),
                            )
                        nc.any.tensor_copy(out=dstT[:, mc, t2 * 512:(t2 + 1) * 512], in_=ps)
            # V natural: [tok, 256] per half, interleaved into per-head [V1h|V2h]
            for t in range(TQ):
                for w_s, x_s, off in ((wv1_s, xv1_s, 0), (wv2_s, xv2_s, DKH)):
                    ps = psmm.tile([P, NH * DKH], F32, tag="mm")
                    for kc in range(KC):
                        nc.tensor.matmul(
                            ps,
                            lhsT=x_s[:, kc, t * P:(t + 1) * P],
                            rhs=w_s[:, kc, :],
                            start=(kc == 0),
                            stop=(kc == KC - 1),
                        )
                    dst_v = vc_s[:, t, :].rearrange("p (h d) -> p h d", d=DV)[:, :, off:off + DKH]
                    src_v = ps.rearrange("p (h d) -> p h d", d=DKH)
                    nc.any.tensor_copy(out=dst_v, in_=src_v)

            # ---- phase 2: attention per head ----
            for h in range(NH if stage >= 2 else 0):
                mc, prow = h // 4, 32 * (h % 4)
                ptb = pt_pool.tile([P, TQ, S], BF16, tag="ptb")  # P^T: [k-part, ktile, q]
                for qt in range(TQ):
                    kext = (qt + 1) * P if causal else S
                    lhsT_q = qt_s[prow:prow + 32, mc, qt * P:(qt + 1) * P]
                    e_t = work.tile([P, S], F32, tag="E")
                    nchunk = (kext + 511) // 512
                    s_parts = stat.tile([P, 2], F32, tag="sparts")
                    for ci in range(nchunk):
                        ks = ci * 512
                        kw = min(512, kext - ks)
                        sps = psS.tile([P, 512], F32, tag="S")
                        # diagonal (causal-masked) block is the last 128 cols
                        kreg = kw - P if causal and ci == nchunk - 1 else kw
                        if kreg > 0:
                            nc.tensor.matmul(
                                sps[:, :kreg],
                                lhsT=lhsT_q,
                                rhs=kt_s[prow:prow + 32, mc, ks:ks + kreg],
                                start=True,
                                stop=True,
                                tile_position=(prow, 0),
                            )
                        if causal and ci == nchunk - 1:
                            nc.tensor.matmul(
                                sps[:, kreg:kw],
                                lhsT=lhsT_q,
                                rhs=kt_s[prow:prow + 32, mc, ks + kreg:ks + kw],
                                start=True,
                                stop=False,
                                tile_position=(prow, 0),
                            )
                            nc.tensor.matmul(
                                sps[:, kreg:kw],
                                lhsT=identb,
                                rhs=cmask,
                                start=False,
                                stop=True,
                            )
                        if not causal:
                            nc.vector.tensor_tensor(
                                out=sps[:, :kw],
                                in0=sps[:, :kw],
                                in1=am_s[:, qt, ks:ks + kw],
                                op=mybir.AluOpType.add,
                            )
                        nc.scalar.activation(
                            out=e_t[:, ks:ks + kw],
                            in_=sps[:, :kw],
                            func=mybir.ActivationFunctionType.Exp,
                            scale=INV_SQRT_DKH,
                            accum_out=s_parts[:, ci:ci + 1],
                        )
                    r_t = stat.tile([P, 1], F32, tag="recip")
                    if nchunk > 1:
                        s_sum = stat.tile([P, 1], F32, tag="ssum")
                        nc.vector.tensor_tensor(
                            out=s_sum, in0=s_parts[:, 0:1], in1=s_parts[:, 1:2],
                            op=mybir.AluOpType.add,
                        )
                    else:
                        s_sum = s_parts[:, 0:1]
                    nc.vector.reciprocal(r_t, s_sum)
                    pr_t = work.tile([P, S], F32, tag="P")
                    nc.gpsimd.tensor_scalar_mul(pr_t[:, :kext], e_t[:, :kext], r_t)
                    nc.sync.dma_start(
                        p_out[h, qt * P:(qt + 1) * P, :kext], pr_t[:, :kext]
                    )
                    # normalized P in bf16 (DVE 2x mode) for the transpose+AV path
                    pb_t = work.tile([P, S], BF16, tag="Pb")
                    nc.vector.tensor_scalar_mul(pb_t[:, :kext], e_t[:, :kext], r_t)
                    # transpose P into ptb, up to 8 blocks per bf16 PSUM tile,
                    # one wide evac copy per qt
                    nkb = (qt + 1 if causal else TQ) if stage >= 3 else 0
                    if nkb > 0:
                        trp = psTr.tile([P, S], BF16, tag="mmb")
                        for kb in range(nkb):
                            nc.tensor.transpose(
                                trp[:, kb * P:(kb + 1) * P],
                                pb_t[:, kb * P:(kb + 1) * P],
                                identb,
                            )
                        src = trp[:, :nkb * P].rearrange("p (b q) -> p b q", q=P)
                        dst = ptb[:, 0:nkb, qt * P:(qt + 1) * P]
                        nc.any.tensor_copy(out=dst, in_=src)
                # AV: O^T[64, q] accumulated over k tiles; feeds qkvT directly
                for qc in range(2 if stage >= 3 else 0):
                    q0 = qc * 512
                    ot = psmm.tile([64, 512], F32, tag="mm")
                    if causal:
                        kts = [kt for kt in range(TQ) if kt * P < q0 + 512]
                    else:
                        kts = list(range(TQ))
                    for i, kt in enumerate(kts):
                        qoff = max(q0, kt * P) if causal else q0
                        nc.tensor.matmul(
                            ot[:, qoff - q0:],
                            lhsT=vc_s[:, kt, h * DV:(h + 1) * DV],
                            rhs=ptb[:, kt, qoff:q0 + 512],
                            start=(i == 0),
                            stop=(i == len(kts) - 1),
                        )
                    nc.any.tensor_copy(
                        out=qkvT_s[64 * (h % 2):64 * (h % 2) + 64, h // 2, q0:q0 + 512],
                        in_=ot,
                    )

            # ---- phase 3: fc + residual + layernorm ----
            # loads deferred here so the startup DMA burst only carries
            # tensors the projection/attention phases actually need
            wfc_s = load_T(wfc, "wfc_s", HALF)
            res_s = const.tile([P, TQ, HALF], F32, tag="res_s")
            for t in range(TQ):
                nc.sync.dma_start(res_s[:, t], res.rearrange("(t p) f -> p t f", p=P)[:, t])
            if not unit_ln:
                g_bc = const.tile([P, HALF], F32, tag="g_bc")
                b_bc = const.tile([P, HALF], F32, tag="b_bc")
                nc.sync.dma_start(g_bc, lng[:, :])
                nc.sync.dma_start(b_bc, lnb[:, :])
            if stage < 4:
                # still produce "out" so outputs are all written
                for t in range(TQ):
                    o_t = ln_pool.tile([P, HALF], F32, tag="o")
                    nc.vector.tensor_copy(out=o_t, in_=res_s[:, t, :])
                    nc.sync.dma_start(out[t * P:(t + 1) * P, :], o_t)
            for t in range(TQ if stage >= 4 else 0):
                fps = psmm.tile([P, HALF], F32, tag="mm")
                for kc in range(KC):
                    nc.tensor.matmul(
                        fps,
                        lhsT=qkvT_s[:, kc, t * P:(t + 1) * P],
                        rhs=wfc_s[:, kc, :],
                        start=(kc == 0),
                        stop=(kc == KC - 1),
                    )
                x_t = ln_pool.tile([P, HALF], F32, tag="x")
                sum_t = stat.tile([P, 1], F32, tag="lnsum")
                if risky_ops:
                    # one DVE pass: x = fps + res, sum_t = rowsum(x)
                    nc.vector.scalar_tensor_tensor(
                        out=x_t, in0=fps, scalar=1.0, in1=res_s[:, t, :],
                        op0=mybir.AluOpType.mult, op1=mybir.AluOpType.add,
                        accum_out=sum_t,
                    )
                else:
                    nc.vector.tensor_tensor(out=x_t, in0=fps, in1=res_s[:, t, :], op=mybir.AluOpType.add)
                    nc.vector.tensor_reduce(
                        out=sum_t, in_=x_t, axis=mybir.AxisListType.X, op=mybir.AluOpType.add
                    )
                sq_t = ln_pool.tile([P, HALF], F32, tag="sq")
                ssq_t = stat.tile([P, 1], F32, tag="lnssq")
                nc.scalar.activation(
                    out=sq_t,
                    in_=x_t,
                    func=mybir.ActivationFunctionType.Square,
                    accum_out=ssq_t,
                )
                mu_t = stat.tile([P, 1], F32, tag="mu")
                nc.scalar.mul(mu_t, sum_t, 1.0 / HALF)
                musq_t = stat.tile([P, 1], F32, tag="musq")
                nc.vector.tensor_tensor(out=musq_t, in0=mu_t, in1=mu_t, op=mybir.AluOpType.mult)
                var_t = stat.tile([P, 1], F32, tag="var")
                nc.vector.tensor_scalar(var_t, ssq_t, 1.0 / HALF, None, mybir.AluOpType.mult)
                nc.vector.tensor_tensor(out=var_t, in0=var_t, in1=musq_t, op=mybir.AluOpType.subtract)
                sd_t = stat.tile([P, 1], F32, tag="sd")
                nc.scalar.activation(
                    out=sd_t, in_=var_t,
                    func=mybir.ActivationFunctionType.Sqrt, bias=eps_t[:, :],
                )
                rstd_t = stat.tile([P, 1], F32, tag="rstd")
                nc.vector.reciprocal(rstd_t, sd_t)
                o_t = ln_pool.tile([P, HALF], F32, tag="o")
                y_dst = o_t if unit_ln else ln_pool.tile([P, HALF], F32, tag="y")
                if risky_ops:
                    nc.vector.tensor_scalar(
                        y_dst, x_t, mu_t, rstd_t,
                        mybir.AluOpType.subtract, mybir.AluOpType.mult,
                    )
                else:
                    nc.vector.tensor_scalar(y_dst, x_t, mu_t, None, mybir.AluOpType.subtract)
                    nc.vector.tensor_scalar_mul(y_dst, y_dst, rstd_t)
                if not unit_ln:
                    nc.vector.tensor_tensor(out=o_t, in0=y_dst, in1=g_bc, op=mybir.AluOpType.mult)
                    nc.vector.tensor_tensor(out=o_t, in0=o_t, in1=b_bc, op=mybir.AluOpType.add)
                nc.sync.dma_start(out[t * P:(t + 1) * P, :], o_t)

    nc.finalize()
    return nc


_nc_lock = threading.Lock()
_nc_cache: dict[tuple, bass.Bass] = {}


def _get_nc(causal: bool, unit_ln: bool = False) -> bass.Bass:
    key = (causal, unit_ln)
    with _nc_lock:
        if key not in _nc_cache:
            _nc_cache[key] = _build_nc(causal, unit_ln=unit_ln)
        return _nc_cache[key]


def _bf(x):
    return np.ascontiguousarray(np.asarray(x, dtype=np.float32)).astype(NPBF16)


def _prep_in_maps(inputs, causal):
    input_Q = np.asarray(inputs["input_Q"], dtype=np.float32)
    input_K = np.asarray(inputs["input_K"], dtype=np.float32)
    input_V = np.asarray(inputs["input_V"], dtype=np.float32)
    attn_mask = np.asarray(inputs["attn_mask"])
    WQ1 = np.asarray(inputs["WQ1"], dtype=np.float32)
    WK1 = np.asarray(inputs["WK1"], dtype=np.float32)
    WV1 = np.asarray(inputs["WV1"], dtype=np.float32)
    WV2 = np.asarray(inputs["WV2"], dtype=np.float32)
    Wfc1 = np.asarray(inputs["Wfc1"], dtype=np.float32)
    Wfc2 = np.asarray(inputs["Wfc2"], dtype=np.float32)
    ln_g = np.asarray(inputs["ln_g"], dtype=np.float32)
    ln_b = np.asarray(inputs["ln_b"], dtype=np.float32)

    lng = np.ascontiguousarray(np.broadcast_to(ln_g[None, :], (P, HALF)))
    lnb = np.ascontiguousarray(np.broadcast_to(ln_b[None, :], (P, HALF)))
    wfcs = (_bf(Wfc1), _bf(Wfc2))

    in_maps = []
    for c in range(8):
        j, b = c // 4, c % 4
        cols = slice(j * NH * DKH, (j + 1) * NH * DKH)
        m = {
            "xqT": _bf(input_Q[b][:, :HALF].T),
            "xkT": _bf(input_K[b][:, :HALF].T),
            "xv1T": _bf(input_V[b][:, :HALF].T),
            "xv2T": _bf(input_V[b][:, HALF:].T),
            "wq": _bf(WQ1[:, cols]),
            "wk": _bf(WK1[:, cols]),
            "wv1": _bf(WV1[:, cols]),
            "wv2": _bf(WV2[:, cols]),
            "wfc": wfcs[j],
            "res": np.ascontiguousarray(input_Q[b][:, j * HALF:(j + 1) * HALF]),
            "lng": lng,
            "lnb": lnb,
        }
        if not causal:
            m["amask"] = (attn_mask[b].astype(np.float32) * np.float32(NEG)).astype(NPBF16)
        in_maps.append(m)
    return in_maps


def kernel(**inputs):
    attn_mask = np.asarray(inputs["attn_mask"])
    triu = np.triu(np.ones((S, S), dtype=bool), k=1)
    causal = all(np.array_equal(attn_mask[b], triu) for b in range(B))
    unit_ln = bool(
        np.all(np.asarray(inputs["ln_g"], dtype=np.float32) == 1.0)
        and np.all(np.asarray(inputs["ln_b"], dtype=np.float32) == 0.0)
    )

    nc = _get_nc(causal, unit_ln)
    in_maps = _prep_in_maps(inputs, causal)

    results = run_bass_kernel_spmd(nc, in_maps, list(range(8))).results

    output = np.empty((B, S, D), dtype=np.float32)
    softmax_attn = np.empty((B, H, S, S), dtype=np.float32)
    for c in range(8):
        j, b = c // 4, c % 4
        output[b][:, j * HALF:(j + 1) * HALF] = results[c]["out"]
        softmax_attn[b, j * NH:(j + 1) * NH] = results[c]["p_out"]
    return output, softmax_attn
